# revision 7
# baseline (speedup 1.0000x reference)
"""Black-Scholes 'all' pricing on 8 Trainium2 NeuronCores (Bass/Tile).

kernel(S0, K, T, vt) -> [N, 4] float32 (call, put, digital_call, digital_put)
N = 8_388_608; options are sharded contiguously across the 8 cores
(trivially data-parallel), each core processing its 1M elements as a
[128 partitions x 8192] block.

Per-core dataflow (R=0.02, Q=0.01):
    dq  = exp(-Q t), dr = exp(-R t)             [ACT]
    Sq  = S0*dq, Kr = K*dr                      [DVE]
    vtt = vt*t                                  [GPSIMD]
    numer = ln(Sq) - ln(Kr) + 0.5*vtt           [ACT ln + DVE]
    isv = exp(-0.5 ln vtt), sv = exp(0.5 ln vtt)[ACT, outputs in PSUM]
    d1  = numer*isv, d2 = d1 - sv               [DVE, PSUM second operand]
    e1  = erf(d1/sqrt2), e2 = erf(d2/sqrt2)     [ACT]
    call = (0.5e1+0.5)*Sq - (0.5e2+0.5)*Kr      [DVE custom affine-mul]
    put  = call + (Kr - Sq)                     [GPSIMD sub + DVE add]
    dc   = (0.5e2+0.5)*dr, dp = (-0.5e2+0.5)*dr [DVE custom affine-mul]

The four outputs are written with stride-4 access patterns into one
[128, F, 4] SBUF tile, so the interleaved [N, 4] output DMAs out as fully
contiguous rows.

Performance notes:
- ACT activation-table steering: ln MUST come from the `natural_log` set
  (the ln in `natural_log_exp_and_others` is ~16x less accurate and its
  error is amplified by isv=1/sqrt(vt*T), up to 100x). exp uses
  `exp_and_others`, erf `sigmoid_and_others`. ACT work is batched per
  table set in sub-phases over groups of G tiles (ordered with explicit
  same-engine dep edges) to amortize the ~2.7us table loads.
- isv/sv/dq/lnKr live in PSUM so the DVE ops consuming them leave the
  shared DVE/GPSIMD SBUF port free; vtt and pc then run on GPSIMD truly
  in parallel with DVE.
"""
import numpy as np

import concourse.bass as bass
import concourse.tile as tile
from concourse import bacc, mybir
from concourse.bass_utils import run_bass_kernel_spmd
from concourse.dve_ops import AFFINE_MUL_REDUCE
from concourse.tile_rust import add_dep_helper

F32 = mybir.dt.float32
AF = mybir.ActivationFunctionType
OP = mybir.AluOpType

R = 0.02
Q = 0.01
INV_SQRT2 = 0.7071067811865476

N = 8_388_608
NCORES = 8
P = 128
FD = N // NCORES // P  # 8192

_KEEP_SETS = ("exp_and_others", "sigmoid_and_others", "natural_log")
_orig_get_tables = None

_NC = None
LAST_EXEC_NS = None
LAST_TRACE_DIR = None
TRACE = False


def _patch_act_tables():
    """Blank the membership of every activation-table set except the three
    we use (list order preserved, so act_func_set_id indices into
    act_info.json stay valid) so the table-load pass resolves ln/exp/erf
    to the sets we want."""
    global _orig_get_tables
    import concourse.hw_specs as hw_specs
    if _orig_get_tables is None:
        _orig_get_tables = hw_specs.get_activation_tables

        def patched(arch):
            tabs = _orig_get_tables(arch)
            return {
                name: (fns if name in _KEEP_SETS else set())
                for name, fns in tabs.items()
            }

        hw_specs.get_activation_tables = patched
        bacc.get_activation_tables = patched


def build_bs(FD=FD, F=1024, G=2, P=P):
    from contextlib import ExitStack
    assert FD % F == 0
    _patch_act_tables()
    ntiles = FD // F
    nc = bacc.Bacc("TRN2", target_bir_lowering=False, debug=False,
                   num_devices=NCORES)
    s_d = nc.dram_tensor("s0", [P, FD], F32, kind="ExternalInput").ap()
    k_d = nc.dram_tensor("k", [P, FD], F32, kind="ExternalInput").ap()
    t_d = nc.dram_tensor("t", [P, FD], F32, kind="ExternalInput").ap()
    v_d = nc.dram_tensor("vt", [P, FD], F32, kind="ExternalInput").ap()
    o_d = nc.dram_tensor("out", [P, FD * 4], F32, kind="ExternalOutput").ap()
    o_d4 = o_d.rearrange("p (n f c) -> p n f c", f=F, c=4)

    def am(out, in0, in1, s0, s1):
        # out = (in0*s0 + s1) * in1
        nc.vector._custom_dve(AFFINE_MUL_REDUCE, out=out, in0=in0, in1=in1,
                              s0=s0, s1=s1)

    with tile.TileContext(nc, pool_alloc_mode="queue") as tc, ExitStack() as ctx:
        inp = ctx.enter_context(tc.tile_pool(name="inp", bufs=2))
        mida = ctx.enter_context(tc.tile_pool(name="mida", bufs=5))
        midc = ctx.enter_context(tc.tile_pool(name="midc", bufs=4))
        pers = ctx.enter_context(tc.tile_pool(name="pers", bufs=2 * G))
        perss = ctx.enter_context(tc.tile_pool(name="perss", bufs=G + 1))
        midb = ctx.enter_context(tc.tile_pool(name="midb", bufs=6))
        outp = ctx.enter_context(tc.tile_pool(name="outp", bufs=2))
        psA = ctx.enter_context(tc.tile_pool(name="psA", bufs=2, space="PSUM"))
        psB = ctx.enter_context(tc.tile_pool(name="psB", bufs=2, space="PSUM"))

        ngroups = (ntiles + G - 1) // G

        # ACT-stream phase ordering: chain every ACT op of a sub-phase after
        # all ACT ops of the previous sub-phase, so the scheduler cannot
        # interleave different table sets and thrash ACT_TABLE_LOADs.
        prev_phase = []
        cur_phase = []

        def act(*args, **kwargs):
            bi = nc.scalar.activation(*args, **kwargs)
            for p in prev_phase:
                add_dep_helper(bi.ins, p.ins, sync=False,
                               reason="act table phase ordering")
            cur_phase.append(bi)
            return bi

        def end_phase():
            if cur_phase:
                prev_phase[:] = cur_phase
                cur_phase.clear()

        st = {}  # per-tile tensor handles

        def emit_sp3(tiles):
            # (exp_and_others): isv, sv; DVE d1, d2; GPSIMD pc — emitted
            # inside the next group's SP1 phase to share one exp residency.
            for i in tiles:
                z = st[i]
                isv = psA.tile([P, F], F32, tag="pa")
                act(isv[:], z["u"][:], AF.Exp, scale=-0.5)
                sv = psA.tile([P, F], F32, tag="pa")
                act(sv[:], z["u"][:], AF.Exp, scale=0.5)
                d1 = perss.tile([P, F], F32, tag="d1")
                nc.vector.tensor_mul(d1[:], z["numer"][:], isv[:])
                d2 = perss.tile([P, F], F32, tag="d2")
                nc.vector.tensor_sub(d2[:], d1[:], sv[:])
                pc = midb.tile([P, F], F32, tag="mb")
                nc.gpsimd.tensor_sub(pc[:], z["Kr"][:], z["Sq"][:])
                z["d1"], z["d2"], z["pc"] = d1, d2, pc

        def emit_sp4(tiles):
            # (sigmoid_and_others): e1, e2; DVE tail; DMA out
            for i in tiles:
                z = st.pop(i)
                e1 = midb.tile([P, F], F32, tag="mb")
                act(e1[:], z["d1"][:], AF.Erf, scale=INV_SQRT2)
                e2 = midb.tile([P, F], F32, tag="mb")
                act(e2[:], z["d2"][:], AF.Erf, scale=INV_SQRT2)
                t1 = midb.tile([P, F], F32, tag="mb")
                am(t1[:], e1[:], z["Sq"][:], 0.5, 0.5)
                t2 = midb.tile([P, F], F32, tag="mb")
                am(t2[:], e2[:], z["Kr"][:], 0.5, 0.5)
                o4 = outp.tile([P, F, 4], F32, tag="o4")
                nc.vector.tensor_sub(o4[:, :, 0], t1[:], t2[:])
                nc.vector.tensor_add(o4[:, :, 1], o4[:, :, 0], z["pc"][:])
                am(o4[:, :, 2], e2[:], z["dr"][:], 0.5, 0.5)
                am(o4[:, :, 3], e2[:], z["dr"][:], -0.5, 0.5)
                nc.sync.dma_start(o_d4[:, i], o4[:])

        prev_tiles = None
        for g in range(ngroups):
            lo, hi = g * G, min((g + 1) * G, ntiles)
            tiles = range(lo, hi)
            # ---- SP1 (exp_and_others): [prev group isv/sv] + dq, dr ----
            if prev_tiles is not None:
                emit_sp3(prev_tiles)
            for i in tiles:
                sl = slice(i * F, (i + 1) * F)
                s = inp.tile([P, F], F32, tag="s")
                nc.sync.dma_start(s[:], s_d[:, sl])
                k = inp.tile([P, F], F32, tag="k")
                nc.sync.dma_start(k[:], k_d[:, sl])
                t = inp.tile([P, F], F32, tag="t")
                nc.sync.dma_start(t[:], t_d[:, sl])
                v = inp.tile([P, F], F32, tag="v")
                nc.sync.dma_start(v[:], v_d[:, sl])

                dq = psB.tile([P, F], F32, tag="pq")
                act(dq[:], t[:], AF.Exp, scale=-Q)
                dr = pers.tile([P, F], F32, tag="dr")
                act(dr[:], t[:], AF.Exp, scale=-R)
                vtt = mida.tile([P, F], F32, tag="mid")
                nc.gpsimd.tensor_mul(vtt[:], t[:], v[:])
                Sq = pers.tile([P, F], F32, tag="Sq")
                nc.vector.tensor_mul(Sq[:], s[:], dq[:])
                Kr = pers.tile([P, F], F32, tag="Kr")
                nc.vector.tensor_mul(Kr[:], k[:], dr[:])
                st[i] = dict(dr=dr, Sq=Sq, Kr=Kr, vtt=vtt)
            end_phase()
            # ---- erf phase for the previous group ----
            if prev_tiles is not None:
                emit_sp4(prev_tiles)
                end_phase()
            # ---- SP2 (natural_log): lnSq, lnKr, u; DVE b, numer ----
            for i in tiles:
                z = st[i]
                lnSq = mida.tile([P, F], F32, tag="mid")
                act(lnSq[:], z["Sq"][:], AF.Ln)
                lnKr = psB.tile([P, F], F32, tag="pq")
                act(lnKr[:], z["Kr"][:], AF.Ln)
                u = midc.tile([P, F], F32, tag="mc")
                act(u[:], z["vtt"][:], AF.Ln)
                b = mida.tile([P, F], F32, tag="mid")
                nc.vector.tensor_sub(b[:], lnSq[:], lnKr[:])
                numer = midc.tile([P, F], F32, tag="mc")
                nc.vector.scalar_tensor_tensor(
                    numer[:], z["vtt"][:], 0.5, b[:], OP.mult, OP.add)
                z["u"] = u
                z["numer"] = numer
            end_phase()
            prev_tiles = tiles
        # drain the last group
        emit_sp3(prev_tiles)
        end_phase()
        emit_sp4(prev_tiles)
        end_phase()
    nc.compile()
    return nc


def _get_nc():
    global _NC
    if _NC is None:
        _NC = build_bs()
    return _NC


def kernel(S0, K, T, vt):
    global LAST_EXEC_NS, LAST_TRACE_DIR
    nc = _get_nc()
    arrs = {"s0": S0, "k": K, "t": T, "vt": vt}
    shards = []
    for i in range(NCORES):
        sl = slice(i * P * FD, (i + 1) * P * FD)
        shards.append({
            name: np.ascontiguousarray(np.asarray(a[sl], dtype=np.float32)
                                       .reshape(P, FD))
            for name, a in arrs.items()
        })
    kwargs = {}
    if TRACE:
        import tempfile
        LAST_TRACE_DIR = tempfile.mkdtemp(prefix="bs_trace_")
        kwargs = dict(trace=True, tmpdir=LAST_TRACE_DIR)
    res = run_bass_kernel_spmd(nc, shards, core_ids=list(range(NCORES)),
                               **kwargs)
    LAST_EXEC_NS = res.exec_time_ns
    out = np.empty((N, 4), dtype=np.float32)
    for i in range(NCORES):
        sl = slice(i * P * FD, (i + 1) * P * FD)
        out[sl] = res.results[i]["out"].reshape(P * FD, 4)
    return out


# revision 8
# speedup vs baseline: 1.0293x; 1.0293x over previous
"""Black-Scholes 'all' pricing on 8 Trainium2 NeuronCores (Bass/Tile).

kernel(S0, K, T, vt) -> [N, 4] float32 (call, put, digital_call, digital_put)
N = 8_388_608; options are sharded contiguously across the 8 cores
(trivially data-parallel), each core processing its 1M elements as a
[128 partitions x 8192] block.

Per-core dataflow (R=0.02, Q=0.01):
    dq  = exp(-Q t), dr = exp(-R t)             [ACT]
    Sq  = S0*dq, Kr = K*dr                      [DVE]
    vtt = vt*t                                  [GPSIMD]
    numer = ln(Sq) - ln(Kr) + 0.5*vtt           [ACT ln + DVE]
    isv = exp(-0.5 ln vtt), sv = exp(0.5 ln vtt)[ACT, outputs in PSUM]
    d1  = numer*isv, d2 = d1 - sv               [DVE, PSUM second operand]
    e1  = erf(d1/sqrt2), e2 = erf(d2/sqrt2)     [ACT]
    call = (0.5e1+0.5)*Sq - (0.5e2+0.5)*Kr      [DVE custom affine-mul]
    put  = call + (Kr - Sq)                     [GPSIMD sub + DVE add]
    dc   = (0.5e2+0.5)*dr, dp = (-0.5e2+0.5)*dr [DVE custom affine-mul]

The four outputs are written with stride-4 access patterns into one
[128, F, 4] SBUF tile, so the interleaved [N, 4] output DMAs out as fully
contiguous rows.

Performance notes:
- ACT activation-table steering: ln MUST come from the `natural_log` set
  (the ln in `natural_log_exp_and_others` is ~16x less accurate and its
  error is amplified by isv=1/sqrt(vt*T), up to 100x). exp uses
  `exp_and_others`, erf `sigmoid_and_others`. ACT work is batched per
  table set in sub-phases over groups of G tiles (ordered with explicit
  same-engine dep edges) to amortize the ~2.7us table loads.
- isv/sv/dq/lnKr live in PSUM so the DVE ops consuming them leave the
  shared DVE/GPSIMD SBUF port free; vtt and pc then run on GPSIMD truly
  in parallel with DVE.
"""
import numpy as np

import concourse.bass as bass
import concourse.tile as tile
from concourse import bacc, mybir
from concourse.bass_utils import run_bass_kernel_spmd
from concourse.dve_ops import AFFINE_MUL_REDUCE
from concourse.tile_rust import add_dep_helper

F32 = mybir.dt.float32
AF = mybir.ActivationFunctionType
OP = mybir.AluOpType

R = 0.02
Q = 0.01
INV_SQRT2 = 0.7071067811865476

N = 8_388_608
NCORES = 8
P = 128
FD = N // NCORES // P  # 8192

_KEEP_SETS = ("exp_and_others", "sigmoid_and_others", "natural_log")
_orig_get_tables = None

_NC = None
LAST_EXEC_NS = None
LAST_TRACE_DIR = None
TRACE = False


def _patch_act_tables():
    """Blank the membership of every activation-table set except the three
    we use (list order preserved, so act_func_set_id indices into
    act_info.json stay valid) so the table-load pass resolves ln/exp/erf
    to the sets we want."""
    global _orig_get_tables
    import concourse.hw_specs as hw_specs
    if _orig_get_tables is None:
        _orig_get_tables = hw_specs.get_activation_tables

        def patched(arch):
            tabs = _orig_get_tables(arch)
            return {
                name: (fns if name in _KEEP_SETS else set())
                for name, fns in tabs.items()
            }

        hw_specs.get_activation_tables = patched
        bacc.get_activation_tables = patched


def build_bs(FD=FD, F=1024, G=2, P=P):
    from contextlib import ExitStack
    assert FD % F == 0
    _patch_act_tables()
    ntiles = FD // F
    nc = bacc.Bacc("TRN2", target_bir_lowering=False, debug=False,
                   num_devices=NCORES)
    s_d = nc.dram_tensor("s0", [P, FD], F32, kind="ExternalInput").ap()
    k_d = nc.dram_tensor("k", [P, FD], F32, kind="ExternalInput").ap()
    t_d = nc.dram_tensor("t", [P, FD], F32, kind="ExternalInput").ap()
    v_d = nc.dram_tensor("vt", [P, FD], F32, kind="ExternalInput").ap()
    o_d = nc.dram_tensor("out", [P, FD * 4], F32, kind="ExternalOutput").ap()
    o_d4 = o_d.rearrange("p (n f c) -> p n f c", f=F, c=4)

    def am(out, in0, in1, s0, s1):
        # out = (in0*s0 + s1) * in1
        nc.vector._custom_dve(AFFINE_MUL_REDUCE, out=out, in0=in0, in1=in1,
                              s0=s0, s1=s1)

    with tile.TileContext(nc) as tc, ExitStack() as ctx:
        inp = ctx.enter_context(tc.tile_pool(name="inp", bufs=2))
        mida = ctx.enter_context(tc.tile_pool(name="mida", bufs=5))
        midc = ctx.enter_context(tc.tile_pool(name="midc", bufs=4))
        pers = ctx.enter_context(tc.tile_pool(name="pers", bufs=2 * G))
        perss = ctx.enter_context(tc.tile_pool(name="perss", bufs=G + 1))
        midb = ctx.enter_context(tc.tile_pool(name="midb", bufs=6))
        outp = ctx.enter_context(tc.tile_pool(name="outp", bufs=2))
        psA = ctx.enter_context(tc.tile_pool(name="psA", bufs=2, space="PSUM"))
        psB = ctx.enter_context(tc.tile_pool(name="psB", bufs=2, space="PSUM"))

        ngroups = (ntiles + G - 1) // G

        # ACT-stream phase ordering: chain every ACT op of a sub-phase after
        # all ACT ops of the previous sub-phase, so the scheduler cannot
        # interleave different table sets and thrash ACT_TABLE_LOADs.
        prev_phase = []
        cur_phase = []

        def act(*args, **kwargs):
            bi = nc.scalar.activation(*args, **kwargs)
            for p in prev_phase:
                add_dep_helper(bi.ins, p.ins, sync=False,
                               reason="act table phase ordering")
            cur_phase.append(bi)
            return bi

        def end_phase():
            if cur_phase:
                prev_phase[:] = cur_phase
                cur_phase.clear()

        st = {}  # per-tile tensor handles

        def emit_sp3(tiles):
            # (exp_and_others): isv, sv; DVE d1, d2; GPSIMD pc — emitted
            # inside the next group's SP1 phase to share one exp residency.
            for i in tiles:
                z = st[i]
                isv = psA.tile([P, F], F32, tag="pa")
                act(isv[:], z["u"][:], AF.Exp, scale=-0.5)
                sv = psA.tile([P, F], F32, tag="pa")
                act(sv[:], z["u"][:], AF.Exp, scale=0.5)
                d1 = perss.tile([P, F], F32, tag="d1")
                nc.vector.tensor_mul(d1[:], z["numer"][:], isv[:])
                d2 = perss.tile([P, F], F32, tag="d2")
                nc.vector.tensor_sub(d2[:], d1[:], sv[:])
                pc = midb.tile([P, F], F32, tag="mb")
                h = F // 2
                nc.gpsimd.tensor_sub(pc[:, :h], z["Kr"][:, :h], z["Sq"][:, :h])
                nc.gpsimd.tensor_sub(pc[:, h:], z["Kr"][:, h:], z["Sq"][:, h:])
                z["d1"], z["d2"], z["pc"] = d1, d2, pc

        def emit_sp4(tiles):
            # (sigmoid_and_others): e1, e2; DVE tail; DMA out
            for i in tiles:
                z = st.pop(i)
                e1 = midb.tile([P, F], F32, tag="mb")
                act(e1[:], z["d1"][:], AF.Erf, scale=INV_SQRT2)
                e2 = midb.tile([P, F], F32, tag="mb")
                act(e2[:], z["d2"][:], AF.Erf, scale=INV_SQRT2)
                t1 = midb.tile([P, F], F32, tag="mb")
                am(t1[:], e1[:], z["Sq"][:], 0.5, 0.5)
                t2 = midb.tile([P, F], F32, tag="mb")
                am(t2[:], e2[:], z["Kr"][:], 0.5, 0.5)
                o4 = outp.tile([P, F, 4], F32, tag="o4")
                nc.vector.tensor_sub(o4[:, :, 0], t1[:], t2[:])
                nc.vector.tensor_add(o4[:, :, 1], o4[:, :, 0], z["pc"][:])
                am(o4[:, :, 2], e2[:], z["dr"][:], 0.5, 0.5)
                am(o4[:, :, 3], e2[:], z["dr"][:], -0.5, 0.5)
                nc.sync.dma_start(o_d4[:, i], o4[:])

        prev_tiles = None
        for g in range(ngroups):
            lo, hi = g * G, min((g + 1) * G, ntiles)
            tiles = range(lo, hi)
            # ---- SP1 (exp_and_others): [prev group isv/sv] + dq, dr ----
            if prev_tiles is not None:
                emit_sp3(prev_tiles)
            for i in tiles:
                sl = slice(i * F, (i + 1) * F)
                s = inp.tile([P, F], F32, tag="s")
                nc.sync.dma_start(s[:], s_d[:, sl])
                k = inp.tile([P, F], F32, tag="k")
                nc.sync.dma_start(k[:], k_d[:, sl])
                t = inp.tile([P, F], F32, tag="t")
                nc.sync.dma_start(t[:], t_d[:, sl])
                v = inp.tile([P, F], F32, tag="v")
                nc.sync.dma_start(v[:], v_d[:, sl])

                dq = psB.tile([P, F], F32, tag="pq")
                act(dq[:], t[:], AF.Exp, scale=-Q)
                dr = pers.tile([P, F], F32, tag="dr")
                act(dr[:], t[:], AF.Exp, scale=-R)
                vtt = mida.tile([P, F], F32, tag="mid")
                h = F // 2
                nc.gpsimd.tensor_mul(vtt[:, :h], t[:, :h], v[:, :h])
                nc.gpsimd.tensor_mul(vtt[:, h:], t[:, h:], v[:, h:])
                Sq = pers.tile([P, F], F32, tag="Sq")
                nc.vector.tensor_mul(Sq[:], s[:], dq[:])
                Kr = pers.tile([P, F], F32, tag="Kr")
                nc.vector.tensor_mul(Kr[:], k[:], dr[:])
                st[i] = dict(dr=dr, Sq=Sq, Kr=Kr, vtt=vtt)
            end_phase()
            # ---- erf phase for the previous group ----
            if prev_tiles is not None:
                emit_sp4(prev_tiles)
                end_phase()
            # ---- SP2 (natural_log): lnSq, lnKr, u; DVE b, numer ----
            for i in tiles:
                z = st[i]
                lnSq = mida.tile([P, F], F32, tag="mid")
                act(lnSq[:], z["Sq"][:], AF.Ln)
                lnKr = psB.tile([P, F], F32, tag="pq")
                act(lnKr[:], z["Kr"][:], AF.Ln)
                u = midc.tile([P, F], F32, tag="mc")
                act(u[:], z["vtt"][:], AF.Ln)
                b = mida.tile([P, F], F32, tag="mid")
                nc.vector.tensor_sub(b[:], lnSq[:], lnKr[:])
                numer = midc.tile([P, F], F32, tag="mc")
                nc.vector.scalar_tensor_tensor(
                    numer[:], z["vtt"][:], 0.5, b[:], OP.mult, OP.add)
                z["u"] = u
                z["numer"] = numer
            end_phase()
            prev_tiles = tiles
        # drain the last group
        emit_sp3(prev_tiles)
        end_phase()
        emit_sp4(prev_tiles)
        end_phase()
    nc.compile()
    return nc


def _get_nc():
    global _NC
    if _NC is None:
        _NC = build_bs()
    return _NC


def kernel(S0, K, T, vt):
    global LAST_EXEC_NS, LAST_TRACE_DIR
    nc = _get_nc()
    arrs = {"s0": S0, "k": K, "t": T, "vt": vt}
    shards = []
    for i in range(NCORES):
        sl = slice(i * P * FD, (i + 1) * P * FD)
        shards.append({
            name: np.ascontiguousarray(np.asarray(a[sl], dtype=np.float32)
                                       .reshape(P, FD))
            for name, a in arrs.items()
        })
    kwargs = {}
    if TRACE:
        import tempfile
        LAST_TRACE_DIR = tempfile.mkdtemp(prefix="bs_trace_")
        kwargs = dict(trace=True, tmpdir=LAST_TRACE_DIR)
    res = run_bass_kernel_spmd(nc, shards, core_ids=list(range(NCORES)),
                               **kwargs)
    LAST_EXEC_NS = res.exec_time_ns
    out = np.empty((N, 4), dtype=np.float32)
    for i in range(NCORES):
        sl = slice(i * P * FD, (i + 1) * P * FD)
        out[sl] = res.results[i]["out"].reshape(P * FD, 4)
    return out
